# revision 1
# baseline (speedup 1.0000x reference)
"""GQA forward (B=2,S=2048,E=2048,H=16,G=4,D=128) on 8 TRN2 NeuronCores.

Sharding: core = (batch b, kv-group g), b=core//4, g=core%4. Each core
computes its group's 4 query heads end-to-end (QKV proj + RoPE + causal
attention + partial out-projection over its 512 Wo rows). Host sums the 4
partial outputs per batch and adds the bias (the unshard step of the
row-parallel out-projection).

Device dataflow (bf16 matmuls, fp32 PSUM):
  - x^T, weight shards, RoPE tables are prepared host-side (bf16).
  - Wq/Wk columns are permuted evens-first within each 128-col head block so
    RoPE's pair rotation becomes a fixed +-64-partition offset in the
    [d, seq] layout: out[0:64] = u*cos[0:64] - (u*sin)[64:128], etc.
  - Scores are computed transposed, ST[k,q] = K'^T-chunk x Q'-tile, so the
    exp'd tile PT[k,q] is directly the moving operand of the P@V matmul
    (OT[d,q] += V_chunk.T @ PT) -- no on-chip transpose of P.
  - Softmax has no max-subtraction (scaled scores are bounded ~ +-17 here);
    denominators come from a ones-column matmul accumulated alongside PV.
  - Causal mask: 4 static 0/1 tiles multiplied into PT (DVE 4x mode) on the
    4 diagonal-crossing k-chunks of each q-tile.
"""

import numpy as np
import ml_dtypes

B, S, E = 2, 2048, 2048
H, G = 16, 4
D = E // H            # 128 head dim
M = H // G            # 4 heads per group
DQ = M * D            # 512 per-core Q columns
QT = 512              # q tile (moving dim)
KC = 128              # k chunk (partition dim of ST)
NEC = E // 128        # 16 contraction chunks
NQT = S // QT         # 4 q tiles
SCALE = 1.0 / float(np.sqrt(D))

_CACHE = {}


def _build_module():
    import concourse.tile as tile
    import concourse.mybir as mybir
    from concourse import bacc
    from contextlib import ExitStack

    bf = mybir.dt.bfloat16
    f32 = mybir.dt.float32
    AF = mybir.ActivationFunctionType

    nc = bacc.Bacc("TRN2", target_bir_lowering=False, debug=False)

    xT = nc.dram_tensor("xT", [E, S], bf, kind="ExternalInput").ap()
    wq = nc.dram_tensor("wq", [E, DQ], bf, kind="ExternalInput").ap()
    wk = nc.dram_tensor("wk", [E, D], bf, kind="ExternalInput").ap()
    wv = nc.dram_tensor("wv", [E, D], bf, kind="ExternalInput").ap()
    wo = nc.dram_tensor("wo", [DQ, E], bf, kind="ExternalInput").ap()
    cos_d = nc.dram_tensor("cos_t", [D, S], bf, kind="ExternalInput").ap()
    sin_d = nc.dram_tensor("sin_t", [D, S], bf, kind="ExternalInput").ap()
    masks_d = nc.dram_tensor("masks", [4, D, QT], bf, kind="ExternalInput").ap()
    out_d = nc.dram_tensor("out", [S, E], bf, kind="ExternalOutput").ap()

    with tile.TileContext(nc) as tc, ExitStack() as ctx:
        singles = ctx.enter_context(tc.tile_pool(name="singles", bufs=1))
        ropep = ctx.enter_context(tc.tile_pool(name="ropep", bufs=2))
        ptp = ctx.enter_context(tc.tile_pool(name="ptp", bufs=4))
        finp = ctx.enter_context(tc.tile_pool(name="finp", bufs=2))
        ystp = ctx.enter_context(tc.tile_pool(name="ystp", bufs=2))
        psA = ctx.enter_context(tc.tile_pool(name="psA", bufs=3, space="PSUM"))
        psB = ctx.enter_context(tc.tile_pool(name="psB", bufs=2, space="PSUM"))
        psS = ctx.enter_context(tc.tile_pool(name="psS", bufs=2, space="PSUM"))

        # ---- resident SBUF tensors ----
        xt = singles.tile([128, NEC, S], bf, tag="xt")          # [e%128, e//128, s]
        wq_s = singles.tile([128, NEC, DQ], bf, tag="wq")
        wk_s = singles.tile([128, NEC, D], bf, tag="wk")
        wv_s = singles.tile([128, NEC, D], bf, tag="wv")
        wo_s = singles.tile([128, M, E], bf, tag="wo")          # [o%128, head, e]
        cos_s = singles.tile([128, S], bf, tag="cos")
        sin_s = singles.tile([128, S], bf, tag="sin")
        mask_s = singles.tile([128, 4, QT], bf, tag="mask")
        kt_s = singles.tile([128, S], bf, tag="kt")             # roped K^T [d, k]
        v_s = singles.tile([128, NEC, D], bf, tag="v")          # V natural [k%128, k//128, d]
        qt_s = singles.tile([128, M, S], bf, tag="qt")          # roped Q^T [d, h, q]
        ot_s = singles.tile([128, M, S], bf, tag="ot")          # normalized O^T [d, h, q]
        ones_col = singles.tile([128, 1], bf, tag="ones_col")   # sums lhsT
        ones_row = singles.tile([1, 128], bf, tag="ones_row")   # bcast lhsT

        nc.vector.memset(ones_col, 1.0)
        nc.vector.memset(ones_row, 1.0)

        # ---- input DMAs ----
        for ec in range(NEC):
            nc.sync.dma_start(out=xt[:, ec, :], in_=xT[ec * 128:(ec + 1) * 128, :])
        nc.sync.dma_start(out=wq_s, in_=wq.rearrange("(n p) d -> p n d", p=128))
        nc.sync.dma_start(out=wk_s, in_=wk.rearrange("(n p) d -> p n d", p=128))
        nc.sync.dma_start(out=wv_s, in_=wv.rearrange("(n p) d -> p n d", p=128))
        nc.sync.dma_start(out=wo_s, in_=wo.rearrange("(h p) e -> p h e", p=128))
        nc.sync.dma_start(out=cos_s, in_=cos_d)
        nc.sync.dma_start(out=sin_s, in_=sin_d)
        nc.sync.dma_start(out=mask_s, in_=masks_d.rearrange("m p j -> p m j"))

        def rope(psum, pos0, n, out_bf):
            """psum [128, n] (evens-first d-layout) -> roped bf16 out_bf.

            sin_s is SIGNED: rows 0:64 hold -sin, rows 64:128 hold +sin, so
            out = psum*cos + swap_halves(psum)*sin_s. The half-swap happens in
            the PSUM-side read of the multiply (mixed PSUM+SBUF operands may
            have different base partitions; SBUF+SBUF may not)."""
            mc = ropep.tile([128, QT], f32, tag="mc")
            ms = ropep.tile([128, QT], f32, tag="ms")
            nc.vector.tensor_mul(mc[:, :n], psum, cos_s[:, pos0:pos0 + n])
            nc.vector.tensor_mul(ms[0:64, :n], psum[64:128], sin_s[0:64, pos0:pos0 + n])
            nc.vector.tensor_mul(ms[64:128, :n], psum[0:64], sin_s[64:128, pos0:pos0 + n])
            nc.vector.tensor_add(out_bf, mc[:, :n], ms[:, :n])

        # ---- K^T (roped) ----
        for kt in range(NQT):
            k0 = kt * QT
            kp = psA.tile([128, QT], f32, tag="mm")
            for ec in range(NEC):
                nc.tensor.matmul(kp, lhsT=wk_s[:, ec, :], rhs=xt[:, ec, k0:k0 + QT],
                                 start=(ec == 0), stop=(ec == NEC - 1))
            rope(kp, k0, QT, kt_s[:, k0:k0 + QT])

        # ---- V (natural [k, d]) ----
        for kc in range(NEC):
            vp = psA.tile([128, D], f32, tag="mm")
            for ec in range(NEC):
                nc.tensor.matmul(vp, lhsT=xt[:, ec, kc * 128:(kc + 1) * 128],
                                 rhs=wv_s[:, ec, :],
                                 start=(ec == 0), stop=(ec == NEC - 1))
            nc.scalar.copy(v_s[:, kc, :], vp)

        # ---- per q-tile: Q^T for all heads, then attention, then out-proj ----
        for qt in range(NQT):
            q0 = qt * QT
            for h in range(M):
                qp = psA.tile([128, QT], f32, tag="mm")
                for ec in range(NEC):
                    nc.tensor.matmul(qp, lhsT=wq_s[:, ec, h * D:(h + 1) * D],
                                     rhs=xt[:, ec, q0:q0 + QT],
                                     start=(ec == 0), stop=(ec == NEC - 1))
                rope(qp, q0, QT, qt_s[:, h, q0:q0 + QT])

            for h in range(M):
                otp = psB.tile([128, QT], f32, tag="ot")
                smp = psS.tile([1, QT], f32, tag="sums")
                nkc = (q0 + QT) // KC
                for kc in range(nkc):
                    stp = psA.tile([128, QT], f32, tag="mm")
                    nc.tensor.matmul(stp, lhsT=kt_s[:, kc * KC:(kc + 1) * KC],
                                     rhs=qt_s[:, h, q0:q0 + QT],
                                     start=True, stop=True)
                    pt = ptp.tile([128, QT], bf, tag="pt")
                    nc.scalar.activation(pt, stp, AF.Exp, scale=SCALE)
                    dm = kc - qt * (QT // KC)
                    if dm >= 0:
                        nc.vector.tensor_mul(pt, pt, mask_s[:, dm, :])
                    nc.tensor.matmul(otp, lhsT=v_s[:, kc, :], rhs=pt,
                                     start=(kc == 0), stop=(kc == nkc - 1))
                    nc.tensor.matmul(smp, lhsT=ones_col, rhs=pt,
                                     start=(kc == 0), stop=(kc == nkc - 1))
                # normalize: ot_s[:, h, q0:q0+QT] = otp * (1/smp) broadcast
                rec = finp.tile([1, QT], f32, tag="rec")
                nc.vector.reciprocal(rec, smp)
                recb = finp.tile([1, QT], bf, tag="recb")
                nc.scalar.copy(recb, rec)
                rbp = psA.tile([128, QT], f32, tag="mm")
                nc.tensor.matmul(rbp, lhsT=ones_row, rhs=recb, start=True, stop=True)
                rb = finp.tile([128, QT], bf, tag="rb")
                nc.scalar.copy(rb, rbp)
                nc.vector.tensor_mul(ot_s[:, h, q0:q0 + QT], otp, rb)

            # out-projection for this q-tile's four 128-row s-chunks
            for sc in range(QT // 128):
                s0 = q0 + sc * 128
                yst = ystp.tile([128, E], bf, tag="yst")
                for et in range(E // QT):
                    yp = psA.tile([128, QT], f32, tag="mm")
                    for h in range(M):
                        nc.tensor.matmul(yp, lhsT=ot_s[:, h, s0:s0 + 128],
                                         rhs=wo_s[:, h, et * QT:(et + 1) * QT],
                                         start=(h == 0), stop=(h == M - 1))
                    nc.scalar.copy(yst[:, et * QT:(et + 1) * QT], yp)
                nc.sync.dma_start(out=out_d[s0:s0 + 128, :], in_=yst)

    nc.compile()
    return nc


def get_module():
    if "nc" not in _CACHE:
        _CACHE["nc"] = _build_module()
    return _CACHE["nc"]


def host_prep(x, Wq, Wk, Wv):
    """Build the 8 per-core input dicts (bf16)."""
    bf = ml_dtypes.bfloat16
    x = np.ascontiguousarray(np.asarray(x, np.float32))

    def perm_cols(W):
        W = np.asarray(W, np.float32).copy()
        for h0 in range(0, W.shape[1], D):
            blk = W[:, h0:h0 + D]
            W[:, h0:h0 + D] = np.concatenate([blk[:, ::2], blk[:, 1::2]], 1)
        return W

    Wq_p = perm_cols(Wq).astype(bf)
    Wk_p = perm_cols(Wk).astype(bf)
    Wv_b = np.asarray(Wv, np.float32).astype(bf)

    inv = 1000.0 ** (-2.0 * np.arange(D // 2, dtype=np.float32) / D)
    ang = np.arange(S, dtype=np.float32)[:, None] * inv[None, :]
    cos_e = np.cos(ang).T
    sin_e = np.sin(ang).T
    cos_t = np.ascontiguousarray(np.concatenate([cos_e, cos_e], 0).astype(bf))
    # signed sin table: rows 0:64 = -sin (for the "even - odd*sin" half),
    # rows 64:128 = +sin (see kernel rope())
    sin_t = np.ascontiguousarray(np.concatenate([-sin_e, sin_e], 0).astype(bf))

    j = np.arange(QT)[None, :]
    p = np.arange(128)[:, None]
    masks = np.stack([(j - p - 128 * m >= 0) for m in range(4)]).astype(bf)
    masks = np.ascontiguousarray(masks)

    xT_b = [np.ascontiguousarray(x[b].T).astype(bf) for b in range(B)]
    return xT_b, Wq_p, Wk_p, Wv_b, cos_t, sin_t, masks


def _ensure_ntff_hook():
    """The agent image's `antenv` lacks `axon_hooks`, so trn_boot silently
    skipped registering the NTFF profile hook. Recreate the registry module
    and register the ctypes-based hook so trace=True works under axon."""
    import sys
    import types
    try:
        from antenv import axon_hooks  # noqa: F401
        return
    except ImportError:
        pass
    import antenv
    mod = types.ModuleType("antenv.axon_hooks")
    _h = [None]
    mod.set_axon_ntff_profile_hook = lambda h: _h.__setitem__(0, h)
    mod.get_axon_ntff_profile_hook = lambda: _h[0]
    sys.modules["antenv.axon_hooks"] = mod
    antenv.axon_hooks = mod
    try:
        from trn_agent_boot.trn_boot import _ntff_profile_via_ctypes
        hook = _ntff_profile_via_ctypes("/opt/axon/libaxon_pjrt.so")
        mod.set_axon_ntff_profile_hook(hook)
    except Exception as e:  # degrade to no-trace
        print("ntff hook registration failed:", e)


def run(inputs, trace=False, trace_cores=None):
    from concourse import bass_utils
    if trace:
        _ensure_ntff_hook()

    x = np.asarray(inputs["x"], np.float32)
    Wo = np.asarray(inputs["Wo"], np.float32)
    bo = np.asarray(inputs["bo"], np.float32)
    bf = ml_dtypes.bfloat16

    xT_b, Wq_p, Wk_p, Wv_b, cos_t, sin_t, masks = host_prep(
        x, inputs["Wq"], inputs["Wk"], inputs["Wv"])

    in_maps = []
    for core in range(8):
        b, g = divmod(core, 4)
        in_maps.append(dict(
            xT=xT_b[b],
            wq=np.ascontiguousarray(Wq_p[:, g * DQ:(g + 1) * DQ]),
            wk=np.ascontiguousarray(Wk_p[:, g * D:(g + 1) * D]),
            wv=np.ascontiguousarray(Wv_b[:, g * D:(g + 1) * D]),
            wo=np.ascontiguousarray(Wo[g * DQ:(g + 1) * DQ, :].astype(bf)),
            cos_t=cos_t, sin_t=sin_t, masks=masks,
        ))

    nc = get_module()
    kw = {}
    if trace:
        kw = dict(trace=True,
                  trace_cores=trace_cores if trace_cores is not None else [0])
    res = bass_utils.run_bass_kernel_spmd(nc, in_maps, core_ids=list(range(8)), **kw)

    out = np.empty((B, S, E), np.float32)
    for b in range(B):
        acc = np.zeros((S, E), np.float32)
        for g in range(G):
            acc += np.asarray(res.results[4 * b + g]["out"], dtype=np.float32)
        out[b] = acc + bo[None, :]
    return out, res


def kernel(**inputs):
    out, _ = run(inputs, trace=False)
    return out



# revision 2
# speedup vs baseline: 1.0002x; 1.0002x over previous
"""GQA forward (B=2,S=2048,E=2048,H=16,G=4,D=128) on 8 TRN2 NeuronCores, v2.

Sharding: core = (batch b, kv-group g), b=core//4, g=core%4. Each core
computes its group's 4 query heads end-to-end (QKV proj + RoPE + causal
attention + partial out-projection over its 512 Wo rows). Host sums the 4
partial outputs per batch and adds the bias.

v2 changes vs baseline (trace-driven):
  - Causal-restricted moving ranges: scores/exp/PV/sums only process
    q-columns j >= 128*(kc-4qt) -> ~15% fewer PE rows; mask shrinks to one
    static [128,128] triangle applied to the 4-head pt tile in one strided
    DVE multiply.
  - Softmax denominators accumulate into ONE [4,512] PSUM tile (one-hot
    [128,4] stationaries), reciprocal via reciprocal_approx_fast (~0.9us vs
    16 x 3.3us), broadcast to 128 partitions with a [4,128] one-row-hot
    stationary matmul.
  - Out-proj PSUM tiles -> bf16 SBUF via DVE copies (scalar engine freed for
    exp), DMA'd out with partition-split descriptors.
  - Host preps all tensors in final device layout (contiguous per-partition
    lines -> fat DMA descriptors), DMAs ordered weights-first.
  - Emission interleaves Q-proj(qt+1) between attn(qt) and normalize(qt) so
    the PE never idles on the softmax-normalize dependency chain.
"""

import numpy as np
import ml_dtypes

B, S, E = 2, 2048, 2048
H, G = 16, 4
D = E // H            # 128 head dim
M = H // G            # 4 heads per group
DQ = M * D            # 512 per-core Q columns
QT = 512              # q tile
KC = 128              # k chunk
NEC = E // 128        # 16 contraction chunks
NQT = S // QT         # 4 q tiles
SCALE = 1.0 / float(np.sqrt(D))

_CACHE = {}


def _build_module():
    import concourse.tile as tile
    import concourse.mybir as mybir
    from concourse import bacc
    from contextlib import ExitStack

    bf = mybir.dt.bfloat16
    f32 = mybir.dt.float32
    AF = mybir.ActivationFunctionType

    nc = bacc.Bacc("TRN2", target_bir_lowering=False, debug=False)

    xT = nc.dram_tensor("xT", [128, NEC, S], bf, kind="ExternalInput").ap()
    wq = nc.dram_tensor("wq", [128, NEC, DQ], bf, kind="ExternalInput").ap()
    wk = nc.dram_tensor("wk", [128, NEC, D], bf, kind="ExternalInput").ap()
    wv = nc.dram_tensor("wv", [128, NEC, D], bf, kind="ExternalInput").ap()
    wo = nc.dram_tensor("wo", [128, M, E], bf, kind="ExternalInput").ap()
    cos_d = nc.dram_tensor("cos_t", [D, S], bf, kind="ExternalInput").ap()
    sin_d = nc.dram_tensor("sin_t", [D, S], bf, kind="ExternalInput").ap()
    tri_d = nc.dram_tensor("tri", [128, M, KC], bf, kind="ExternalInput").ap()
    sel_d = nc.dram_tensor("sel", [128, M, M], bf, kind="ExternalInput").ap()
    rsel_d = nc.dram_tensor("rsel", [M, M, 128], bf, kind="ExternalInput").ap()
    out_d = nc.dram_tensor("out", [S, E], bf, kind="ExternalOutput").ap()

    with tile.TileContext(nc) as tc, ExitStack() as ctx:
        singles = ctx.enter_context(tc.tile_pool(name="singles", bufs=1))
        ropep = ctx.enter_context(tc.tile_pool(name="ropep", bufs=2))
        ptp = ctx.enter_context(tc.tile_pool(name="ptp", bufs=6))
        otsp = ctx.enter_context(tc.tile_pool(name="otsp", bufs=2))
        ystp = ctx.enter_context(tc.tile_pool(name="ystp", bufs=2))
        recp = ctx.enter_context(tc.tile_pool(name="recp", bufs=2))
        rbpool = ctx.enter_context(tc.tile_pool(name="rbpool", bufs=2))
        psMM = ctx.enter_context(tc.tile_pool(name="psMM", bufs=3, space="PSUM"))
        psOT = ctx.enter_context(tc.tile_pool(name="psOT", bufs=4, space="PSUM"))
        psSM = ctx.enter_context(tc.tile_pool(name="psSM", bufs=1, space="PSUM"))

        # ---- resident SBUF tensors ----
        xt = singles.tile([128, NEC, S], bf, tag="xt")
        wq_s = singles.tile([128, NEC, DQ], bf, tag="wq")
        wk_s = singles.tile([128, NEC, D], bf, tag="wk")
        wv_s = singles.tile([128, NEC, D], bf, tag="wv")
        wo_s = singles.tile([128, M, E], bf, tag="wo")
        cos_s = singles.tile([128, S], bf, tag="cos")
        sin_s = singles.tile([128, S], bf, tag="sin")
        tri_s = singles.tile([128, M, KC], bf, tag="tri")
        sel_s = singles.tile([128, M, M], bf, tag="sel")
        rsel_s = singles.tile([M, M, 128], bf, tag="rsel")
        kt_s = singles.tile([128, S], bf, tag="kt")             # roped K^T [d, k]
        v_s = singles.tile([128, NEC, D], bf, tag="v")          # V natural [k%128, k//128, d]
        qt_s = singles.tile([128, M, S], bf, tag="qt")          # roped Q^T [d, h, q]
        scratch = singles.tile([1, 2], bf, tag="scratch")

        # ---- input DMAs (weights first; everything partition-split so no
        # single queue serializes >3us; xt chunked so proj MMs stream) ----
        def split_dma(dst, src, ways):
            step = 128 // ways
            for i in range(ways):
                nc.sync.dma_start(out=dst[i * step:(i + 1) * step],
                                  in_=src[i * step:(i + 1) * step])

        split_dma(wk_s, wk, 4)
        # early xt chunks arrive fine-split so the K-proj accumulation can
        # start ~3us in; later chunks stay coarse (sync dispatch ~350ns each)
        for ec in range(4):
            ways = 4 if ec < 2 else 2
            step = 128 // ways
            for i in range(ways):
                nc.sync.dma_start(out=xt[i * step:(i + 1) * step, ec, :],
                                  in_=xT[i * step:(i + 1) * step, ec, :])
        split_dma(cos_s, cos_d, 2)
        split_dma(sin_s, sin_d, 2)
        nc.sync.dma_start(out=tri_s, in_=tri_d)
        nc.sync.dma_start(out=sel_s, in_=sel_d)
        nc.sync.dma_start(out=rsel_s, in_=rsel_d)
        nc.sync.dma_start(out=wv_s, in_=wv)
        for ec in range(4, NEC):
            nc.sync.dma_start(out=xt[:, ec, :], in_=xT[:, ec, :])
        nc.sync.dma_start(out=wq_s, in_=wq)
        nc.sync.dma_start(out=wo_s, in_=wo)

        # preload the Exp activation table during the DMA window
        nc.scalar.activation(scratch[0:1, 0:1], sel_s[0:1, 0, 0:1], AF.Exp)

        def rope(psum, pos0, out_bf):
            """psum [128, QT] (evens-first d-layout) -> roped bf16 out_bf.
            sin_s rows 0:64 hold -sin, rows 64:128 hold +sin."""
            mc = ropep.tile([128, QT], bf, tag="mc")
            ms = ropep.tile([128, QT], bf, tag="ms")
            nc.vector.tensor_mul(mc, psum, cos_s[:, pos0:pos0 + QT])
            nc.vector.tensor_mul(ms[0:64, :], psum[64:128], sin_s[0:64, pos0:pos0 + QT])
            nc.vector.tensor_mul(ms[64:128, :], psum[0:64], sin_s[64:128, pos0:pos0 + QT])
            nc.vector.tensor_add(out_bf, mc, ms)

        # ---- K^T (roped) + first V tiles, ec-major: each arriving xt chunk
        # unblocks 7 matmuls, filling the PE during the input-DMA window ----
        kps = [psOT.tile([128, QT], f32, tag="ot", name=f"kp{kt}")
               for kt in range(NQT)]
        vps = [psMM.tile([128, QT], f32, tag="mm", name=f"vp{kc}")
               for kc in range(3)]
        for ec in range(NEC):
            for kt in range(NQT):
                nc.tensor.matmul(kps[kt], lhsT=wk_s[:, ec, :],
                                 rhs=xt[:, ec, kt * QT:(kt + 1) * QT],
                                 start=(ec == 0), stop=(ec == NEC - 1))
            for kc in range(3):
                nc.tensor.matmul(vps[kc][:, 0:D],
                                 lhsT=xt[:, ec, kc * 128:(kc + 1) * 128],
                                 rhs=wv_s[:, ec, :],
                                 start=(ec == 0), stop=(ec == NEC - 1))
        for kt in range(NQT):
            rope(kps[kt], kt * QT, kt_s[:, kt * QT:(kt + 1) * QT])
        for kc in range(3):
            nc.scalar.copy(v_s[:, kc, :], vps[kc][:, 0:D])

        # ---- remaining V (natural [k, d]) ----
        for kc in range(3, NEC):
            vp = psMM.tile([128, QT], f32, tag="mm")
            for ec in range(NEC):
                nc.tensor.matmul(vp[:, 0:D], lhsT=xt[:, ec, kc * 128:(kc + 1) * 128],
                                 rhs=wv_s[:, ec, :],
                                 start=(ec == 0), stop=(ec == NEC - 1))
            nc.scalar.copy(v_s[:, kc, :], vp[:, 0:D])

        def q_proj(qt):
            q0 = qt * QT
            for h in range(M):
                qp = psMM.tile([128, QT], f32, tag="mm")
                for ec in range(NEC):
                    nc.tensor.matmul(qp, lhsT=wq_s[:, ec, h * D:(h + 1) * D],
                                     rhs=xt[:, ec, q0:q0 + QT],
                                     start=(ec == 0), stop=(ec == NEC - 1))
                rope(qp, q0, qt_s[:, h, q0:q0 + QT])

        q_proj(0)
        for qt in range(NQT):
            q0 = qt * QT
            nkc = (q0 + QT) // KC

            # ---- attention for this q tile (4 heads batched per k chunk) ----
            otp = [psOT.tile([128, QT], f32, tag="ot", name=f"otp{h}")
                   for h in range(M)]
            smp4 = psSM.tile([M, QT], f32, tag="sm")
            deferred = []
            for kc in range(nkc):
                dm = kc - qt * (QT // KC)
                j0 = 128 * dm if dm > 0 else 0
                pt4 = ptp.tile([128, M, QT], bf, tag="pt")
                for h in range(M):
                    stp = psMM.tile([128, QT], f32, tag="mm")
                    nc.tensor.matmul(stp[:, j0:QT],
                                     lhsT=kt_s[:, kc * KC:(kc + 1) * KC],
                                     rhs=qt_s[:, h, q0 + j0:q0 + QT],
                                     start=True, stop=True)
                    nc.scalar.activation(pt4[:, h, j0:QT], stp[:, j0:QT],
                                         AF.Exp, scale=SCALE)
                if dm >= 0:
                    nc.vector.tensor_mul(pt4[:, :, j0:j0 + KC],
                                         pt4[:, :, j0:j0 + KC], tri_s)
                # sums before PV; the last two chunks' PV matmuls are emitted
                # after the final sums so the PE stays busy while the
                # reciprocal chain runs on Vector/Scalar
                for h in range(M):
                    nc.tensor.matmul(smp4[:, j0:QT], lhsT=sel_s[:, h, :],
                                     rhs=pt4[:, h, j0:QT],
                                     start=(kc == 0 and h == 0),
                                     stop=(kc == nkc - 1 and h == M - 1))
                if kc >= nkc - 4 and nkc > 4:
                    deferred.append((kc, j0, pt4))
                    continue
                for h in range(M):
                    nc.tensor.matmul(otp[h][:, j0:QT], lhsT=v_s[:, kc, :],
                                     rhs=pt4[:, h, j0:QT],
                                     start=(kc == 0), stop=(kc == nkc - 1))
            for kc, j0, pt4 in deferred:
                for h in range(M):
                    nc.tensor.matmul(otp[h][:, j0:QT], lhsT=v_s[:, kc, :],
                                     rhs=pt4[:, h, j0:QT],
                                     start=(kc == 0), stop=(kc == nkc - 1))

            # ---- normalize: ot_s[:, h, :] = otp[h] / den_h ----
            rec4 = recp.tile([M, QT], f32, tag="rec")
            nc.vector.reciprocal_approx_fast(rec4, smp4)
            recb4 = recp.tile([M, QT], bf, tag="recb")
            nc.scalar.copy(recb4, rec4)
            ot_s = otsp.tile([128, M, QT], bf, tag="ots")
            for h in range(M):
                rbp = psMM.tile([128, QT], f32, tag="mm")
                nc.tensor.matmul(rbp, lhsT=rsel_s[:, h, :], rhs=recb4,
                                 start=True, stop=True)
                rb = rbpool.tile([128, QT], bf, tag="rb")
                nc.vector.tensor_copy(rb, rbp)
                nc.vector.tensor_mul(ot_s[:, h, :], otp[h], rb)

            # Q-proj of the next tile fills the PE behind the normalize chain
            if qt + 1 < NQT:
                q_proj(qt + 1)

            # ---- out-projection for this q-tile ----
            for sc in range(QT // 128):
                s0 = q0 + sc * 128
                yst = ystp.tile([128, M, QT], bf, tag="yst")
                for et in range(E // QT):
                    yp = psMM.tile([128, QT], f32, tag="mm")
                    for h in range(M):
                        nc.tensor.matmul(yp, lhsT=ot_s[:, h, sc * 128:(sc + 1) * 128],
                                         rhs=wo_s[:, h, et * QT:(et + 1) * QT],
                                         start=(h == 0), stop=(h == M - 1))
                    nc.vector.tensor_copy(yst[:, et, :], yp)
                # full-row slab DMAs (4KB descriptors), partition-split so
                # four queues drain one s-chunk in parallel
                for i in range(4):
                    nc.sync.dma_start(
                        out=out_d[s0 + i * 32:s0 + (i + 1) * 32, :],
                        in_=yst[i * 32:(i + 1) * 32, :, :])

    nc.compile()
    return nc


def get_module():
    if "nc" not in _CACHE:
        _CACHE["nc"] = _build_module()
    return _CACHE["nc"]


def host_prep(x, Wq, Wk, Wv, Wo):
    """Build per-core device-layout arrays (bf16, contiguous partition lines)."""
    bf = ml_dtypes.bfloat16
    x = np.ascontiguousarray(np.asarray(x, np.float32))

    def perm_cols(W):
        W = np.asarray(W, np.float32).copy()
        for h0 in range(0, W.shape[1], D):
            blk = W[:, h0:h0 + D]
            W[:, h0:h0 + D] = np.concatenate([blk[:, ::2], blk[:, 1::2]], 1)
        return W

    def chunked(W):  # [E, C] -> [128, NEC, C]
        C = W.shape[1]
        return np.ascontiguousarray(
            W.reshape(NEC, 128, C).transpose(1, 0, 2).astype(bf))

    Wq_p = perm_cols(Wq)
    Wk_p = perm_cols(Wk)
    Wv_f = np.asarray(Wv, np.float32)
    Wo_f = np.asarray(Wo, np.float32)

    inv = 1000.0 ** (-2.0 * np.arange(D // 2, dtype=np.float32) / D)
    ang = np.arange(S, dtype=np.float32)[:, None] * inv[None, :]
    cos_e = np.cos(ang).T
    sin_e = np.sin(ang).T
    cos_t = np.ascontiguousarray(np.concatenate([cos_e, cos_e], 0).astype(bf))
    sin_t = np.ascontiguousarray(np.concatenate([-sin_e, sin_e], 0).astype(bf))

    # causal triangle for a diagonal 128x128 block, replicated per head
    jj = np.arange(KC)[None, :]
    p = np.arange(128)[:, None]
    tri = (jj >= p).astype(np.float32)
    tri4 = np.ascontiguousarray(
        np.repeat(tri[:, None, :], M, axis=1).astype(bf))      # [128, M, 128]

    # sums stationary: sel[k, h, j] = (j == h)
    sel = np.zeros((128, M, M), np.float32)
    for h in range(M):
        sel[:, h, h] = 1.0
    sel = np.ascontiguousarray(sel.astype(bf))

    # reciprocal-broadcast stationary: rsel[k, h, j] = (k == h)
    rsel = np.zeros((M, M, 128), np.float32)
    for h in range(M):
        rsel[h, h, :] = 1.0
    rsel = np.ascontiguousarray(rsel.astype(bf))

    xT_b = []
    for b in range(B):
        xt = x[b].T  # [E, S]
        xT_b.append(np.ascontiguousarray(
            xt.reshape(NEC, 128, S).transpose(1, 0, 2).astype(bf)))

    per_g = {}
    for g in range(G):
        wo_g = Wo_f[g * DQ:(g + 1) * DQ, :]  # [512, E]
        per_g[g] = dict(
            wq=chunked(Wq_p[:, g * DQ:(g + 1) * DQ]),
            wk=chunked(Wk_p[:, g * D:(g + 1) * D]),
            wv=chunked(Wv_f[:, g * D:(g + 1) * D]),
            wo=np.ascontiguousarray(
                wo_g.reshape(M, 128, E).transpose(1, 0, 2).astype(bf)),
        )
    return xT_b, per_g, cos_t, sin_t, tri4, sel, rsel


def _ensure_ntff_hook():
    """The agent image's `antenv` lacks `axon_hooks`; recreate the registry
    module and register the ctypes-based hook so trace=True works."""
    import sys
    import types
    try:
        from antenv import axon_hooks  # noqa: F401
        return
    except ImportError:
        pass
    import antenv
    mod = types.ModuleType("antenv.axon_hooks")
    _h = [None]
    mod.set_axon_ntff_profile_hook = lambda h: _h.__setitem__(0, h)
    mod.get_axon_ntff_profile_hook = lambda: _h[0]
    sys.modules["antenv.axon_hooks"] = mod
    antenv.axon_hooks = mod
    try:
        from trn_agent_boot.trn_boot import _ntff_profile_via_ctypes
        hook = _ntff_profile_via_ctypes("/opt/axon/libaxon_pjrt.so")
        mod.set_axon_ntff_profile_hook(hook)
    except Exception as e:
        print("ntff hook registration failed:", e)


def run(inputs, trace=False, trace_cores=None):
    from concourse import bass_utils
    if trace:
        _ensure_ntff_hook()

    x = np.asarray(inputs["x"], np.float32)
    bo = np.asarray(inputs["bo"], np.float32)

    xT_b, per_g, cos_t, sin_t, tri4, sel, rsel = host_prep(
        x, inputs["Wq"], inputs["Wk"], inputs["Wv"], inputs["Wo"])

    in_maps = []
    for core in range(8):
        b, g = divmod(core, 4)
        in_maps.append(dict(
            xT=xT_b[b], cos_t=cos_t, sin_t=sin_t, tri=tri4, sel=sel, rsel=rsel,
            **per_g[g],
        ))

    nc = get_module()
    kw = {}
    if trace:
        kw = dict(trace=True,
                  trace_cores=trace_cores if trace_cores is not None else [0])
    res = bass_utils.run_bass_kernel_spmd(nc, in_maps, core_ids=list(range(8)), **kw)

    out = np.empty((B, S, E), np.float32)
    for b in range(B):
        acc = np.zeros((S, E), np.float32)
        for g in range(G):
            acc += np.asarray(res.results[4 * b + g]["out"], dtype=np.float32)
        out[b] = acc + bo[None, :]
    return out, res


def kernel(**inputs):
    out, _ = run(inputs, trace=False)
    return out
